# revision 147
# baseline (speedup 1.0000x reference)
"""Trainium2 Bass kernel for a pre-LN transformer block (B=2, T=2048, D=1024,
NH=16, HD=64, DFF=4096) on 8 NeuronCores.

Sharding: 4 cores per batch. Core j of a batch owns query tiles
{j, 4+j, 8+j, 12+j} (one per "slot" s=0..3), so every core does identical
causal-attention work: slot s attends to key tiles 0..4s+3 (40 score tiles
per head instead of 64). The host permutes each batch's tokens so the core's
own tile of group g sits at group position 4g+3; the key set covered by slot
s is unchanged, and causality within the last 4 key tiles of each slot is
applied via a small per-core data mask.

Precision: every matmul (K/V/Q/scores/att@V/proj/fc1/fc2) runs in fp8e4m3
with DoubleRow perf mode. Scores get DoubleRow despite hd=64 by splitting
the head dim 2x32 across partitions: the host permutes W_q/W_k output
columns so the K/Q evacuations land in a [32-sub-head | 2-chan-tile]
layout (kT8/qT8), and each score matmul reads a 32-partition slice with an
explicit tile_position (K carries 1/4 and Q 4x to stay in fp8 normal
range). att weights are the fp8 exp output; the softmax denominator is an
fp8 ones-column in v_aug so numerator and denominator round consistently.
fp32 PSUM accumulation everywhere.

Scheduling: ln1 runs channel-major via PE ones-matmul stats (ln-chain
intermediates parked in spare 32-aligned rows of the stats PSUM bank), with
the fp8 normalize pass split DVE/Pool. Slot post-processing (proj, ln2 with
a bit-trick+Newton rsqrt on DVE - no Sqrt table switch) is sliced into 16
pieces spread across the next slot's head loop; slot 0 pairs two heads per
score-PSUM tile so one Exp covers both. fc1 weights prefetch during phase
A; fc2's DMA overlaps fc1's matmuls; the fc2 tail fuses
residual+bias+unscale into one scalar_tensor_tensor per tile. The residual
stream is carried as 16*x so the proj/fc2 weight scale (16) cancels in
that fused tail.
"""

import sys

for _p in ("/opt/trn_rl_repo", "/root/.axon_site/_ro/trn_rl_repo"):
    if _p not in sys.path:
        sys.path.insert(0, _p)

import numpy as np
import ml_dtypes

import concourse.bass as bass
import concourse.tile as tile
from concourse import bacc, mybir
from concourse.bass_utils import run_bass_kernel_spmd

B = 2
T = 2048
D = 1024
NH = 16
HD = 64
DFF = 4 * D
EPS = 1e-5
P = 128
KO = D // P            # 8 contraction tiles over D
KP = KO // 2           # 4 DoubleRow pairs over D
N_CORES = 8
CPB = N_CORES // B     # cores per batch
TC = T // CPB          # 512 own tokens per core
NS = 4                 # query slots per core
NKT = T // P           # 16 key tiles
NFT = DFF // P         # 32 dff tiles
FP = NFT // 2          # 16 DoubleRow pairs over DFF
FC = 512
NCH = T // FC          # 4 token chunks for ln1
CS = [4, 8, 12, 16]    # key tiles per slot
TK = sum(CS)           # 40 score tiles per head

f32 = mybir.dt.float32
f32r = mybir.dt.float32r
bf16 = mybir.dt.bfloat16
fp8 = mybir.dt.float8e4
AF = mybir.ActivationFunctionType
ALU = mybir.AluOpType
DR = mybir.MatmulPerfMode.DoubleRow

_CACHE = {}


def build_nc():
    nc = bacc.Bacc("TRN2", target_bir_lowering=False)

    io = {}
    d = nc.declare_dram_parameter
    io["xbT"] = d("xbT", [D, T], bf16, isOutput=False)          # permuted, ch-major
    io["xbT8"] = d("xbT8", [D, T], fp8, isOutput=False)         # fp8 copy, stats only
    io["x_own16"] = d("x_own16", [TC, D], f32, isOutput=False)  # 16*(x_own+bvproj)
    io["w_k8"] = d("w_k8", [P, KO, KP, 2, P], fp8, isOutput=False)
    io["w_q8"] = d("w_q8", [P, KO, KP, 2, P], fp8, isOutput=False)
    io["w_v8"] = d("w_v8", [P, KP, 2, D], fp8, isOutput=False)
    io["w_p8"] = d("w_p8", [P, KP, 2, D], fp8, isOutput=False)
    io["fc1_w8"] = d("fc1_w8", [P, NFT, KP, 2, P], fp8, isOutput=False)
    io["fc2_w8"] = d("fc2_w8", [P, FP, 2, D], fp8, isOutput=False)
    io["b_k"] = d("b_k", [P, KO], f32, isOutput=False)
    io["b_q"] = d("b_q", [P, KO], f32, isOutput=False)
    io["fc1_b"] = d("fc1_b", [P, NFT], f32, isOutput=False)
    io["b2_rep16"] = d("b2_rep16", [P, D], f32, isOutput=False)
    io["mask"] = d("mask", [P, NS, 4, P], bf16, isOutput=False)
    io["identb"] = d("identb", [P, P], bf16, isOutput=False)
    io["out"] = d("out", [TC, D], f32, isOutput=True)

    with tile.TileContext(nc) as tc:
        _emit(nc, tc, io)
    nc.compile()
    return nc


def _emit(nc, tc, io):
    from contextlib import ExitStack

    with ExitStack() as ctx:
        singles = ctx.enter_context(tc.tile_pool(name="singles", bufs=1))

        ones_invD = singles.tile([P, 1], bf16)
        nc.vector.memset(ones_invD, 1.0 / D)
        eps1 = singles.tile([1, 1], f32)
        nc.vector.memset(eps1, EPS)
        epsT = singles.tile([P, 1], f32)
        nc.vector.memset(epsT, EPS * 256.0)          # ln2 runs on 16*x
        identb = singles.tile([P, P], bf16)

        # ---- tiles that live into phases D/E (must be below pMain on the
        # pool stack, so allocated up front) ----
        pLate_cm = tc.tile_pool(name="pLate", bufs=1)
        pLate = pLate_cm.__enter__()
        attn_out = pLate.tile([P, NS, D], bf16)
        attn_outT = pLate.tile([P, KO, TC], fp8)
        X2 = pLate.tile([P, NS, D], f32)
        xn2T = pLate.tile([P, KO, TC], fp8)
        w_p8 = pLate.tile([P, KP, 2, D], fp8)
        # phase-E weights, prefetched during phase A (DMA issued in the
        # chunk loop so the xbT loads go out first); fc2_w8 stays in pE —
        # its DMA overlaps the fc1 matmuls.
        fc1_w8 = pLate.tile([P, NFT, KP, 2, P], fp8)
        fc1_b = pLate.tile([P, NFT], f32)
        mask_sb = pLate.tile([P, NS, 4, P], bf16)

        # ---- persistent big tiles (live through phase C) ----
        pMain_cm = tc.tile_pool(name="pMain", bufs=1)
        pMain = pMain_cm.__enter__()
        kT8 = pMain.tile([P, 2, 4, T], fp8)          # [p, i, hg, t]
        v_aug = pMain.tile([P, NKT, NH, HD + 1], fp8)
        qT8 = pMain.tile([P, 2, 4, TC], fp8)

        # ---------- Phase A: ln1 channel-major, V interleaved ----------
        pA_cm = tc.tile_pool(name="pA", bufs=1)
        pA = pA_cm.__enter__()
        xn8 = pA.tile([P, KO, T], fp8)
        w_v8 = pA.tile([P, KP, 2, D], fp8)
        w_k8 = pA.tile([P, KO, KP, 2, P], fp8)
        w_q8 = pA.tile([P, KO, KP, 2, P], fp8)
        b_k8 = pA.tile([P, KO], f32)
        b_q8 = pA.tile([P, KO], f32)
        nc.vector.memset(v_aug[:, :, :, HD:HD + 1], 1.0)

        with tc.tile_pool(name="lnp", bufs=2) as lnp, \
             tc.tile_pool(name="lns", bufs=1) as lns, \
             tc.tile_pool(name="psSt", bufs=2, space="PSUM") as psSt, \
             tc.tile_pool(name="psK", bufs=3, space="PSUM") as psK, \
             tc.tile_pool(name="psQ", bufs=1, space="PSUM") as psQ, \
             tc.tile_pool(name="psV", bufs=2, space="PSUM") as psV:

            def load_xbT(ch):
                t = lnp.tile([P, KO, FC], bf16, tag="xbT")
                for kg in range(KO // 2):
                    nc.sync.dma_start(
                        out=t[:, 2 * kg:2 * kg + 2, :],
                        in_=bass.AP(tensor=io["xbT"],
                                    offset=(2 * kg * P) * T + ch * FC,
                                    ap=[[T, P], [P * T, 2], [1, FC]]))
                return t

            def emit_k(ct, ch):
                kp_ps = psK.tile([P, FC], f32, tag="k")
                cs = slice(ch * FC, (ch + 1) * FC)
                for kp in range(KP):
                    nc.tensor.matmul(
                        kp_ps, w_k8[:, ct, kp], xn8[:, 2 * kp:2 * kp + 2, cs],
                        start=(kp == 0), stop=(kp == KP - 1), perf_mode=DR)
                nc.scalar.activation(
                    out=kT8[:, ct // 4, ct % 4, cs], in_=kp_ps,
                    func=AF.Identity, bias=b_k8[:, ct:ct + 1],
                    scale=0.03125)

            pre = [load_xbT(0)]
            nc.sync.dma_start(out=w_v8, in_=io["w_v8"].ap())
            pre.append(load_xbT(1))
            nc.sync.dma_start(out=w_q8, in_=io["w_q8"].ap())
            nc.sync.dma_start(out=b_q8, in_=io["b_q"].ap())
            nc.sync.dma_start(out=w_k8, in_=io["w_k8"].ap())
            nc.sync.dma_start(out=b_k8, in_=io["b_k"].ap())
            nc.sync.dma_start(out=identb, in_=io["identb"].ap())
            nc.sync.dma_start(out=w_p8, in_=io["w_p8"].ap())

            def emit_v(tt):
                for vc in range(2):
                    vp = psV.tile([P, FC], f32, tag="vp")
                    for kp in range(KP):
                        nc.tensor.matmul(
                            vp,
                            xn8[:, 2 * kp:2 * kp + 2, tt * P:(tt + 1) * P],
                            w_v8[:, kp, :, vc * FC:(vc + 1) * FC],
                            start=(kp == 0), stop=(kp == KP - 1),
                            perf_mode=DR)
                    nc.scalar.activation(
                        out=v_aug[:, tt, vc * 8:(vc + 1) * 8, 0:HD],
                        in_=vp.rearrange("p (h d) -> p h d", h=8),
                        func=AF.Identity, scale=0.125)

            for ch in range(NCH + 1):
                if ch < NCH:
                    sl = slice(ch * FC, (ch + 1) * FC)
                    xbT = pre[ch]
                    # stage 1a: stats + ln chain (ACT sqrt before stage-2 copies)
                    work = lnp.tile([P, KO, FC], bf16, tag="work")
                    for kg in range(KO // 2):
                        if ch < 2:
                            # ACT is idle at the head of phase A; Square is
                            # in every activation table
                            nc.scalar.activation(
                                out=work[:, 2 * kg:2 * kg + 2, :],
                                in_=xbT[:, 2 * kg:2 * kg + 2, :],
                                func=AF.Square)
                        else:
                            nc.vector.tensor_mul(
                                out=work[:, 2 * kg:2 * kg + 2, :],
                                in0=xbT[:, 2 * kg:2 * kg + 2, :],
                                in1=xbT[:, 2 * kg:2 * kg + 2, :])
                    st = psSt.tile([97, FC], f32, tag="st")
                    s_ps = st[0:1, :]
                    q_ps = st[32:33, :]
                    for ko in range(KO):
                        nc.tensor.matmul(s_ps, ones_invD, xbT[:, ko, :],
                                         start=(ko == 0), stop=(ko == KO - 1))
                    for ko in range(KO):
                        nc.tensor.matmul(q_ps, ones_invD, work[:, ko, :],
                                         start=(ko == 0), stop=(ko == KO - 1))
                    musq = lns.tile([1, FC], bf16, tag="musq")
                    with nc.allow_low_precision(reason="ln1 musq bf16"):
                        nc.scalar.square(out=musq, in_=s_ps)
                    ve = st[96:97, :]
                    nc.vector.tensor_tensor(out=ve, in0=q_ps, in1=musq,
                                            op=ALU.subtract)
                    std = st[64:65, :]
                    nc.scalar.activation(out=std, in_=ve, func=AF.Sqrt,
                                         bias=eps1)
                    if ch == NCH - 1:
                        nc.scalar.activation(out=eps1, in_=eps1, func=AF.Exp)
                        nc.vector.memset(eps1, EPS)
                    rstd = lns.tile([1, FC], bf16, tag="rstd")
                    with nc.allow_low_precision(reason="ln1 rstd bcast bf16"):
                        nc.vector.reciprocal(out=rstd, in_=std)
                    nmr = lns.tile([1, FC], bf16, tag="nmr")
                    nc.vector.tensor_mul(out=nmr, in0=s_ps, in1=rstd)
                    rn = lnp.tile([P, 2, FC], bf16, tag="rn")
                    nc.gpsimd.partition_broadcast(rn[:, 0, :], rstd)
                    nc.gpsimd.partition_broadcast(rn[:, 1, :], nmr)
                if ch >= 1:
                    # stage 2: V + K + Q-slot of the previous chunk
                    s_own = ch - 1
                    qcol = (4 * s_own + 3) * P       # own tile at group pos 3
                    for i in range(4):
                        emit_v((ch - 1) * 4 + i)
                        for ct in (2 * i, 2 * i + 1):
                            emit_k(ct, ch - 1)
                            qp = psQ.tile([P, P], f32, tag="q")
                            for kp in range(KP):
                                nc.tensor.matmul(
                                    qp, w_q8[:, ct, kp],
                                    xn8[:, 2 * kp:2 * kp + 2, qcol:qcol + P],
                                    start=(kp == 0), stop=(kp == KP - 1),
                                    perf_mode=DR)
                            nc.vector.tensor_scalar(
                                out=qT8[:, ct // 4, ct % 4,
                                        s_own * P:(s_own + 1) * P],
                                in0=qp, scalar1=1.0 / 16.0,
                                scalar2=b_q8[:, ct:ct + 1],
                                op0=ALU.mult, op1=ALU.add)
                if ch < NCH:
                    # stage 1b: normalize into fp8
                    for ko in range(KO):
                        nc.vector.tensor_mul(out=work[:, ko, :],
                                             in0=xbT[:, ko, :], in1=rn[:, 0, :])
                    for ko in range(KO):
                        seng = nc.gpsimd if ko % 2 == 0 else nc.vector
                        seng.tensor_tensor(
                            out=xn8[:, ko, sl], in0=work[:, ko, :],
                            in1=rn[:, 1, :], op=ALU.subtract)
                    if ch + 2 < NCH:
                        pre.append(load_xbT(ch + 2))
                    if ch == 1:
                        # phase-E weight prefetch, after the last xbT issue
                        nc.sync.dma_start(out=mask_sb, in_=io["mask"].ap())
                        for g in range(8):
                            nc.sync.dma_start(
                                out=fc1_w8[:, g * 4:(g + 1) * 4],
                                in_=io["fc1_w8"].ap()[:, g * 4:(g + 1) * 4])
                        nc.sync.dma_start(out=fc1_b, in_=io["fc1_b"].ap())

        pA_cm.__exit__(None, None, None)

        # ---------- Phase C: attention, phase-D work interleaved per slot ----------
        pD_cm = tc.tile_pool(name="pD", bufs=1)
        pD = pD_cm.__enter__()
        x_own16 = pD.tile([P, NS, D], f32)
        nc.sync.dma_start(
            out=x_own16,
            in_=bass.AP(tensor=io["x_own16"], offset=0,
                        ap=[[D, P], [D * P, NS], [1, D]]))
        mvall = pD.tile([P, NS, 2], f32)
        rstd4 = pD.tile([P, NS], f32)
        xn2a = pD.tile([P, NS, D], bf16)
        i32 = mybir.dt.int32
        with tc.tile_pool(name="attp", bufs=12) as attp, \
             tc.tile_pool(name="attp2", bufs=7) as attp2, \
             tc.tile_pool(name="dp", bufs=4) as dp, \
             tc.tile_pool(name="psC", bufs=2, space="PSUM") as psC, \
             tc.tile_pool(name="psD", bufs=2, space="PSUM") as psD, \
             tc.tile_pool(name="psDp", bufs=2, space="PSUM") as psDp:

            def emit_piece(s, piece):
                """One slice of slot-s post-processing (proj/ln2), spread
                across the next slot's head loop so the psD single-buffer
                round trips hide under attention."""
                if piece < 8:
                    ko = piece
                    tp = psD.tile([P, P], bf16, tag="tp")
                    nc.tensor.transpose(
                        tp, attn_out[:, s, ko * P:(ko + 1) * P], identb)
                    nc.vector.tensor_copy(
                        out=attn_outT[:, ko, s * P:(s + 1) * P], in_=tp)
                elif piece in (8, 9):
                    oc = piece - 8
                    pj = psDp.tile([P, FC], f32, tag="pj")
                    for kp in range(KP):
                        nc.tensor.matmul(
                            pj,
                            attn_outT[:, 2 * kp:2 * kp + 2, s * P:(s + 1) * P],
                            w_p8[:, kp, :, oc * FC:(oc + 1) * FC],
                            start=(kp == 0), stop=(kp == KP - 1), perf_mode=DR)
                    nc.vector.tensor_add(
                        out=X2[:, s, oc * FC:(oc + 1) * FC], in0=pj,
                        in1=x_own16[:, s, oc * FC:(oc + 1) * FC])
                elif piece == 10:
                    stats = dp.tile([P, 2, 6], f32, tag="st2")
                    nc.vector.bn_stats(out=stats[:, 0, :], in_=X2[:, s, 0:FC])
                    nc.vector.bn_stats(out=stats[:, 1, :], in_=X2[:, s, FC:D])
                    nc.vector.bn_aggr(out=mvall[:, s, :], in_=stats)
                    # rstd via bit-trick seed + 2 Newton steps, all on DVE —
                    # keeps ACT on the Exp table through phase C
                    vv = dp.tile([P, 1], f32, tag="vv")
                    nc.vector.tensor_tensor(out=vv, in0=mvall[:, s, 1:2],
                                            in1=epsT, op=ALU.add)
                    y = rstd4[:, s:s + 1]
                    nc.vector.tensor_scalar(
                        out=y.bitcast(i32), in0=vv.bitcast(i32), scalar1=1,
                        scalar2=None, op0=ALU.logical_shift_right)
                    nc.vector.tensor_scalar(
                        out=y.bitcast(i32), in0=y.bitcast(i32), scalar1=-1,
                        scalar2=None, op0=ALU.bitwise_xor)
                    nc.vector.tensor_scalar(
                        out=y.bitcast(i32), in0=y.bitcast(i32),
                        scalar1=0x5f3759df + 1, scalar2=None, op0=ALU.add)
                    nt = dp.tile([P, 1], f32, tag="nt")
                    for _ in range(2):
                        nc.vector.tensor_tensor(out=nt, in0=y, in1=y,
                                                op=ALU.mult)
                        nc.vector.tensor_tensor(out=nt, in0=nt, in1=vv,
                                                op=ALU.mult)
                        nc.vector.tensor_scalar(out=nt, in0=nt, scalar1=-0.5,
                                                scalar2=1.5, op0=ALU.mult,
                                                op1=ALU.add)
                        nc.vector.tensor_tensor(out=y, in0=y, in1=nt,
                                                op=ALU.mult)
                elif piece == 11:
                    if s == NS - 1:
                        # tail latency matters: split halves across DVE/Pool
                        nc.vector.tensor_scalar(
                            out=xn2a[:, s, 0:FC], in0=X2[:, s, 0:FC],
                            scalar1=mvall[:, s, 0:1],
                            scalar2=rstd4[:, s:s + 1],
                            op0=ALU.subtract, op1=ALU.mult)
                        nc.gpsimd.tensor_scalar(
                            out=xn2a[:, s, FC:D], in0=X2[:, s, FC:D],
                            scalar1=mvall[:, s, 0:1],
                            scalar2=rstd4[:, s:s + 1],
                            op0=ALU.subtract, op1=ALU.mult)
                    else:
                        nc.gpsimd.tensor_scalar(
                            out=xn2a[:, s], in0=X2[:, s, :],
                            scalar1=mvall[:, s, 0:1],
                            scalar2=rstd4[:, s:s + 1],
                            op0=ALU.subtract, op1=ALU.mult)
                else:
                    for ko in (2 * (piece - 12), 2 * (piece - 12) + 1):
                        tp = psD.tile([P, P], bf16, tag="tp")
                        nc.tensor.transpose(
                            tp, xn2a[:, s, ko * P:(ko + 1) * P], identb)
                        nc.vector.tensor_copy(
                            out=xn2T[:, ko, s * P:(s + 1) * P], in_=tp)

            def emit_av_tail(s, h, attTh):
                """mask + att@V + normalize for one head; attTh is [P,CS,P]."""
                nc.vector.tensor_mul(
                    out=attTh[:, CS[s] - 4:CS[s], :],
                    in0=attTh[:, CS[s] - 4:CS[s], :],
                    in1=mask_sb[:, s])
                avt = psDp.tile([P, FC], f32, tag="pj")
                av = avt[:, 0:HD + 1]
                for k2 in range(CS[s] // 2):
                    nc.tensor.matmul(av, attTh[:, 2 * k2:2 * k2 + 2, :],
                                     v_aug[:, 2 * k2:2 * k2 + 2, h, :],
                                     start=(k2 == 0),
                                     stop=(k2 == CS[s] // 2 - 1),
                                     perf_mode=DR)
                recip = dp.tile([P, 1], f32, tag="recip")
                nc.vector.reciprocal(out=recip, in_=av[:, HD:HD + 1])
                nc.vector.tensor_scalar(
                    out=attn_out[:, s, h * HD:(h + 1) * HD],
                    in0=av[:, 0:HD], scalar1=recip, scalar2=None,
                    op0=ALU.mult)

            # slot 0 (CS=4): two heads per score psum tile, one exp per pair
            for h in range(0, NH, 2):
                attT2 = attp.tile([P, 2, 4, P], fp8, tag="attT0")
                sc = psC.tile([P, 2, FC], f32, tag="sc")
                scv = sc.rearrange("p a b -> p (a b)").rearrange(
                    "p (k q) -> p k q", q=P)
                for hh in range(2):
                    hp = 32 * ((h + hh) % 4)
                    hg = (h + hh) // 4
                    qsl = qT8[hp:hp + 32, :, hg, 0:P]
                    for kt in range(4):
                        nc.tensor.matmul(
                            scv[:, 4 * hh + kt, :],
                            kT8[hp:hp + 32, :, hg, kt * P:(kt + 1) * P],
                            qsl, start=True, stop=True,
                            perf_mode=DR, tile_position=(hp, 0))
                nc.scalar.activation(out=attT2, in_=scv, func=AF.Exp)
                for hh in range(2):
                    emit_av_tail(0, h + hh, attT2[:, hh])

            for h in range(NH):
                emit_piece(0, h)
                if h % 2 == 0:
                    # slot 1 (CS=8): one head per iteration, 1 exp each
                    pass
                hp = 32 * (h % 4)
                hg = h // 4
                attT = attp.tile([P, CS[1], P], fp8, tag="attT1")
                qsl = qT8[hp:hp + 32, :, hg, P:2 * P]
                sc = psC.tile([P, 2, FC], f32, tag="sc")
                scv = sc.rearrange("p a b -> p (a b)").rearrange(
                    "p (k q) -> p k q", q=P)
                for kt in range(8):
                    nc.tensor.matmul(
                        scv[:, kt, :],
                        kT8[hp:hp + 32, :, hg, kt * P:(kt + 1) * P],
                        qsl, start=True, stop=True,
                        perf_mode=DR, tile_position=(hp, 0))
                nc.scalar.activation(out=attT, in_=scv, func=AF.Exp)
                emit_av_tail(1, h, attT)

            # slot 2 (CS=12): two heads per iteration, 24 tiles in 3 fills
            for h in range(0, NH, 2):
                emit_piece(1, h)
                emit_piece(1, h + 1)
                attT2p = attp2.tile([P, 2, 12, P], fp8, tag="attT2")
                flat = attT2p.rearrange("p a b q -> p (a b) q")
                for fill in range(3):
                    sc = psC.tile([P, 2, FC], f32, tag="sc")
                    scv = sc.rearrange("p a b -> p (a b)").rearrange(
                        "p (k q) -> p k q", q=P)
                    for i in range(8):
                        g = fill * 8 + i
                        hh, kt = g // 12, g % 12
                        hp = 32 * ((h + hh) % 4)
                        hg = (h + hh) // 4
                        nc.tensor.matmul(
                            scv[:, i, :],
                            kT8[hp:hp + 32, :, hg, kt * P:(kt + 1) * P],
                            qT8[hp:hp + 32, :, hg, 2 * P:3 * P],
                            start=True, stop=True,
                            perf_mode=DR, tile_position=(hp, 0))
                    nc.scalar.activation(
                        out=flat[:, fill * 8:(fill + 1) * 8, :],
                        in_=scv, func=AF.Exp)
                for hh in range(2):
                    emit_av_tail(2, h + hh, attT2p[:, hh])

            for h in range(NH):
                emit_piece(2, h)
                if h >= 2 and h % 2 == 0:
                    # slot-3 attn_out transposes: ko needs heads 2ko..2ko+1
                    emit_piece(NS - 1, h // 2 - 1)
                hp = 32 * (h % 4)
                hg = h // 4
                attT = attp.tile([P, CS[3], P], fp8, tag="attT3")
                qsl = qT8[hp:hp + 32, :, hg, 3 * P:4 * P]
                for seg in range(2):
                    base = seg * 8
                    sc = psC.tile([P, 2, FC], f32, tag="sc")
                    scv = sc.rearrange("p a b -> p (a b)").rearrange(
                        "p (k q) -> p k q", q=P)
                    for i in range(8):
                        kt = base + i
                        nc.tensor.matmul(
                            scv[:, i, :],
                            kT8[hp:hp + 32, :, hg, kt * P:(kt + 1) * P],
                            qsl, start=True, stop=True,
                            perf_mode=DR, tile_position=(hp, 0))
                    nc.scalar.activation(
                        out=attT[:, base:base + 8, :],
                        in_=scv, func=AF.Exp)
                emit_av_tail(3, h, attT)
            for piece in range(7, 16):
                emit_piece(NS - 1, piece)

        pD_cm.__exit__(None, None, None)
        pMain_cm.__exit__(None, None, None)

        # ---------- Phase E: fc1 -> gelu -> hT(fp8); fc2 + final residual ----------
        pE_cm = tc.tile_pool(name="pE", bufs=1)
        pE = pE_cm.__enter__()
        hT = pE.tile([P, NFT, TC], fp8)
        fc2_w8 = pE.tile([P, FP, 2, D], fp8)
        nc.sync.dma_start(out=fc2_w8, in_=io["fc2_w8"].ap())
        b2_rep16 = pE.tile([P, D], f32)
        nc.sync.dma_start(out=b2_rep16, in_=io["b2_rep16"].ap())
        X2b = pE.tile([P, NS, D], f32)

        X2b_decl_done = True
        with tc.tile_pool(name="fe", bufs=3) as fe, \
             tc.tile_pool(name="psF", bufs=4, space="PSUM") as psF:
            for s in range(NS):
                # X2b = X2/16 + b2 (pre-divided so the fc2 tail is one op)
                nc.vector.scalar_tensor_tensor(
                    out=X2b[:, s], in0=X2[:, s], scalar=1.0 / 16.0,
                    in1=b2_rep16, op0=ALU.mult, op1=ALU.add)
            for ft in range(NFT):
                f1 = psF.tile([P, TC], f32, tag="f1")
                for kp in range(KP):
                    nc.tensor.matmul(f1, fc1_w8[:, ft, kp],
                                     xn2T[:, 2 * kp:2 * kp + 2, :],
                                     start=(kp == 0), stop=(kp == KP - 1),
                                     perf_mode=DR)
                nc.scalar.activation(out=hT[:, ft, :], in_=f1, func=AF.Gelu,
                                     bias=fc1_b[:, ft:ft + 1], scale=0.125)
            for s in range(NS):
                for oc in range(2):
                    f2 = psF.tile([P, FC], f32, tag="f2")
                    for fp_ in range(FP):
                        nc.tensor.matmul(
                            f2, hT[:, 2 * fp_:2 * fp_ + 2, s * P:(s + 1) * P],
                            fc2_w8[:, fp_, :, oc * FC:(oc + 1) * FC],
                            start=(fp_ == 0), stop=(fp_ == FP - 1), perf_mode=DR)
                    o = fe.tile([P, FC], f32, tag="o")
                    nc.vector.scalar_tensor_tensor(
                        out=o, in0=f2, scalar=1.0 / 16.0,
                        in1=X2b[:, s, oc * FC:(oc + 1) * FC],
                        op0=ALU.mult, op1=ALU.add)
                    nc.sync.dma_start(
                        out=io["out"].ap()[s * P:(s + 1) * P,
                                           oc * FC:(oc + 1) * FC],
                        in_=o)
        pE_cm.__exit__(None, None, None)
        pLate_cm.__exit__(None, None, None)


def _stage_inputs(x, w_qkv, w_proj, ln1_w, ln1_b, ln2_w, ln2_b,
                  fc1_w, fc1_b, fc2_w, fc2_b):
    """Host-side sharding / ln folding / fp8 pre-scaling / tiling."""
    f = np.float32
    bf = ml_dtypes.bfloat16
    f8 = ml_dtypes.float8_e4m3
    x = np.asarray(x, f)
    w_qkv = np.asarray(w_qkv, f)
    ln1_w, ln1_b = np.asarray(ln1_w, f), np.asarray(ln1_b, f)
    ln2_w, ln2_b = np.asarray(ln2_w, f), np.asarray(ln2_b, f)
    fc1_wf, fc1_bf = np.asarray(fc1_w, f), np.asarray(fc1_b, f)
    fc2_wf, fc2_bf = np.asarray(fc2_w, f), np.asarray(fc2_b, f)
    w_projf = np.asarray(w_proj, f)

    wq_f = ln1_w[:, None] * w_qkv
    bq_f = ln1_b @ w_qkv
    scale = 1.0 / np.sqrt(HD)

    def pack_lhsT(w, mult):
        # [D, M] -> [P, M/P, KP, 2, P]; [p, ct, kp, i, m] = mult*w[(2kp+i)*P+p, ct*P+m]
        Din, M = w.shape
        t = (mult * w).reshape(KP, 2, P, M // P, P)
        return np.ascontiguousarray(t.transpose(2, 3, 0, 1, 4)).astype(f8)

    # output-channel permutation for the fp8 score layout: tile ct' = 4*i + hg
    # holds, at column m = 32*a + b, original channel (4*hg + a)*64 + 32*i + b,
    # so the K/Q evacs land [head-sub a | chan-sub i] split across partitions.
    colperm = np.empty(D, np.int64)
    for ct_ in range(KO):
        i_, hg = ct_ // 4, ct_ % 4
        for m in range(P):
            a, b_ = m // 32, m % 32
            colperm[ct_ * P + m] = (4 * hg + a) * 64 + 32 * i_ + b_

    def pack_rhs(w, mult):
        # [D, N] -> [P, KP, 2, N]
        Din, N = w.shape
        t = (mult * w).reshape(KP, 2, P, N)
        return np.ascontiguousarray(t.transpose(2, 0, 1, 3)).astype(f8)

    w_q8 = pack_lhsT(wq_f[:, 0:D][:, colperm] * scale, 64.0)
    w_k8 = pack_lhsT(wq_f[:, D:2 * D][:, colperm], 8.0)
    w_v8 = pack_rhs(wq_f[:, 2 * D:3 * D], 8.0)
    w_p8 = pack_rhs(w_projf, 16.0)
    # K carries an extra 1/4 and Q an extra 4x so both sit well inside the
    # fp8 normal range; the score product is unchanged.
    b_q_h = np.ascontiguousarray(
        (bq_f[0:D][colperm] * scale * 4.0).reshape(KO, P).T).astype(f)
    b_k_h = np.ascontiguousarray(
        (bq_f[D:2 * D][colperm] * 0.25).reshape(KO, P).T).astype(f)
    b_v = bq_f[2 * D:3 * D]
    bvproj = b_v @ w_projf

    fc1s = ln2_w[:, None] * fc1_wf
    fc1_w8 = np.ascontiguousarray(
        (8.0 * fc1s).reshape(KP, 2, P, NFT, P).transpose(2, 3, 0, 1, 4)).astype(f8)
    fc1_b_h = np.ascontiguousarray(
        (ln2_b @ fc1_wf + fc1_bf).reshape(NFT, P).T).astype(f)
    fc2_w8 = np.ascontiguousarray(
        (16.0 * fc2_wf).reshape(FP, 2, P, D).transpose(2, 0, 1, 3)).astype(f8)
    b2_rep16 = np.ascontiguousarray(
        np.broadcast_to(fc2_bf, (P, D))).astype(f)
    eye = np.eye(P, dtype=f)

    shared = {
        "w_k8": w_k8, "w_q8": w_q8, "w_v8": w_v8, "w_p8": w_p8,
        "fc1_w8": fc1_w8, "fc2_w8": fc2_w8,
        "b_k": b_k_h, "b_q": b_q_h, "fc1_b": fc1_b_h, "b2_rep16": b2_rep16,
        "identb": eye.astype(bf),
    }

    tri = np.tril(np.ones((P, P), np.float32)).T  # [p, q] = 1 iff p <= q
    in_maps = []
    for c in range(N_CORES):
        b = c // CPB
        j = c % CPB
        perm = []
        for g in range(NS):
            others = [4 * g + i for i in range(4) if i != j]
            perm += others + [4 * g + j]
        tok_perm = np.concatenate([np.arange(t * P, (t + 1) * P) for t in perm])
        xp = x[b][tok_perm]
        xbT_c = np.ascontiguousarray(xp.T).astype(bf)
        xbT8_c = xbT_c.astype(f8)
        own_rows = np.concatenate(
            [np.arange((4 * s + j) * P, (4 * s + j + 1) * P) for s in range(NS)])
        x_own16_c = (16.0 * (x[b][own_rows] + bvproj)).astype(f)
        m = np.zeros((P, NS, 4, P), np.float32)
        for s in range(NS):
            for i in range(4):
                kt_abs = perm[4 * s + i]
                if kt_abs < 4 * s + j:
                    m[:, s, i, :] = 1.0
                elif kt_abs == 4 * s + j:
                    m[:, s, i, :] = tri
        im = dict(shared)
        im.update({"xbT": xbT_c, "xbT8": xbT8_c, "x_own16": x_own16_c,
                   "mask": m.astype(bf)})
        in_maps.append(im)
    return in_maps


def kernel(**inputs) -> np.ndarray:
    if "nc" not in _CACHE:
        _CACHE["nc"] = build_nc()
    nc = _CACHE["nc"]
    in_maps = _stage_inputs(**inputs)
    res = run_bass_kernel_spmd(nc, in_maps, list(range(N_CORES)))
    out = np.empty((B, T, D), np.float32)
    for c in range(N_CORES):
        b = c // CPB
        j = c % CPB
        r = res.results[c]["out"]
        for s in range(NS):
            t_abs = 4 * s + j
            out[b, t_abs * P:(t_abs + 1) * P] = r[s * P:(s + 1) * P]
    return out

